# revision 4
# baseline (speedup 1.0000x reference)
"""Causal linear attention (ELU+1 feature map) on 8 trn2 NeuronCores.

Sharding: core i handles batch b=i//2, sequence half h=i%2 (T=2048 -> 1024
tokens/core).  Second-half cores recompute the first half's running state
S0 = sum_tau phi(k_tau) [v_tau, 1]  (128x129, col 128 = z) from k/v of the
first half; first-half cores get zeroed aux inputs so their S0 == 0.  This
avoids any cross-core communication.

Math per core (chunk C=128, 8 own chunks + 8 "pre" state-only chunks):
  phi(y) = min(exp(y), 1) + relu(y)            (== ELU(y)+1 exactly)
  pre_q/pre_k = W_phi @ q/k (feature-major, fp32r matmuls)
  pre_tok(kpre) = token-major via per-chunk stationary kT + rank-1 bias matmul
  A^T_c = K_c Q_c^T ; mask (tau<=t) fused into PSUM->SBUF bf16 copy
  O_c = Amask^T.T @ [V_c, 1] + Q_c @ S_snap    (den accumulates in col 128)
  S  += K_tok_c^T-style matmul with [V_c, 1]   (PSUM accumulation)
  out_c = O_c[:, :128] * (1 / O_c[:, 128])
"""

import numpy as np

B, T, D, DV = 4, 2048, 128, 128
H = T // 2          # tokens per core
C = 128             # chunk
NCH = H // C        # chunks per half
NCORES = 8

_cache = {}


def _build():
    import concourse.bacc as bacc
    import concourse.tile as tile
    from concourse import mybir

    F32 = mybir.dt.float32
    F32R = mybir.dt.float32r
    BF16 = mybir.dt.bfloat16
    AF = mybir.ActivationFunctionType

    nc = bacc.Bacc(None, target_bir_lowering=False, debug=False,
                   num_devices=NCORES)

    qT = nc.declare_dram_parameter("qT", [D, H], F32R, isOutput=False)
    kT = nc.declare_dram_parameter("kT", [D, H], F32R, isOutput=False)
    kTp = nc.declare_dram_parameter("kTp", [D, H], BF16, isOutput=False)
    WT = nc.declare_dram_parameter("WT", [D, D], F32R, isOutput=False)
    WTb = nc.declare_dram_parameter("WTb", [D, D], BF16, isOutput=False)
    bias = nc.declare_dram_parameter("bias", [D, 1], F32, isOutput=False)
    btile = nc.declare_dram_parameter("btile", [1, H], BF16, isOutput=False)
    vaug = nc.declare_dram_parameter("vaug", [NCH, C, DV + 1], BF16,
                                     isOutput=False)
    vaugp = nc.declare_dram_parameter("vaugp", [NCH, C, DV + 1], BF16,
                                      isOutput=False)
    mask = nc.declare_dram_parameter("mask", [C, C], BF16, isOutput=False)
    ident = nc.declare_dram_parameter("ident", [C, C], BF16, isOutput=False)
    out = nc.declare_dram_parameter("out", [H, DV], F32, isOutput=True)

    with tile.TileContext(nc) as tc:
        with (
            tc.tile_pool(name="cst", bufs=1) as cst,
            tc.tile_pool(name="io", bufs=1) as io,
            tc.tile_pool(name="phi", bufs=1) as phip,
            tc.tile_pool(name="wrk", bufs=3) as wrk,
            tc.tile_pool(name="ps_pre", bufs=2, space="PSUM") as ps_pre,
            tc.tile_pool(name="ps_s", bufs=1, space="PSUM") as ps_s,
            tc.tile_pool(name="ps_sm", bufs=3, space="PSUM") as ps_sm,
        ):
            # ---- loads ----
            s_WT = cst.tile([D, D], F32R)
            s_WTb = cst.tile([D, D], BF16)
            s_bias = cst.tile([D, 1], F32)
            s_btile = cst.tile([1, H], BF16)
            s_mask = cst.tile([C, C], BF16)
            s_ident = cst.tile([C, C], BF16)
            s_ones = cst.tile([1, C], BF16)
            nc.sync.dma_start(out=s_WT, in_=WT[:, :])
            nc.sync.dma_start(out=s_WTb, in_=WTb[:, :])
            nc.sync.dma_start(out=s_bias, in_=bias[:, :])
            nc.sync.dma_start(out=s_btile, in_=btile[:, :])
            nc.sync.dma_start(out=s_mask, in_=mask[:, :])
            nc.sync.dma_start(out=s_ident, in_=ident[:, :])
            nc.vector.memset(s_ones, 1.0)

            s_qT = io.tile([D, H], F32R)
            s_kT = io.tile([D, H], F32R)
            s_kTp = io.tile([D, H], BF16)
            s_v = io.tile([C, NCH, DV + 1], BF16)
            s_vp = io.tile([C, NCH, DV + 1], BF16)
            nc.sync.dma_start(out=s_qT, in_=qT[:, :])
            nc.sync.dma_start(out=s_kT, in_=kT[:, :])
            nc.sync.dma_start(out=s_kTp, in_=kTp[:, :])
            nc.sync.dma_start(out=s_v, in_=vaug[:, :, :].rearrange("c p j -> p c j"))
            nc.sync.dma_start(out=s_vp, in_=vaugp[:, :, :].rearrange("c p j -> p c j"))

            # ---- phases B+C: phi matmuls + pointwise ----
            # three pre-activation tensors share a 2-buffer [*, H] PSUM tag
            e_f = phip.tile([D, 2 * H], BF16)
            r_f = phip.tile([D, 2 * H], BF16)
            phi_f = phip.tile([D, 2 * H], BF16)   # [Q^T | K^T]
            for i, src in enumerate((s_qT, s_kT)):
                pre = ps_pre.tile([D, H], F32, tag="pre")
                for j in range(H // 512):
                    nc.tensor.matmul(pre[:, 512 * j:512 * (j + 1)], s_WT,
                                     src[:, 512 * j:512 * (j + 1)],
                                     start=True, stop=True)
                sl = slice(H * i, H * (i + 1))
                nc.scalar.activation(e_f[:, sl], pre, AF.Exp,
                                     bias=s_bias, scale=1.0)
                nc.scalar.activation(r_f[:, sl], pre, AF.Relu,
                                     bias=s_bias, scale=1.0)
                nc.vector.tensor_scalar_min(e_f[:, sl], e_f[:, sl], 1.0)
                nc.vector.tensor_add(phi_f[:, sl], e_f[:, sl], r_f[:, sl])

            pst = ps_pre.tile([C, H], F32, tag="pre")       # token-major pre
            for j in range(H // 512):
                nc.tensor.matmul(pst[:, 512 * j:512 * (j + 1)], s_ones,
                                 s_btile[:, 512 * j:512 * (j + 1)],
                                 start=True, stop=True)
            for c in range(NCH):
                nc.tensor.matmul(pst[:, C * c:C * (c + 1)],
                                 s_kTp[:, C * c:C * (c + 1)], s_WTb,
                                 start=False, stop=True)

            e_t = phip.tile([C, H], BF16)
            r_t = phip.tile([C, H], BF16)
            phi_t = phip.tile([C, H], BF16)       # K_tok_pre chunks
            nc.scalar.activation(e_t, pst, AF.Exp)
            nc.scalar.activation(r_t, pst, AF.Relu)
            nc.vector.tensor_scalar_min(e_t, e_t, 1.0)
            nc.vector.tensor_add(phi_t, e_t, r_t)

            QT = phi_f[:, 0:H]
            KT = phi_f[:, H:2 * H]

            # ---- phase D: transpose own K chunks (feature->token major) ----
            ktok = phip.tile([C, H], BF16)
            for c in range(NCH):
                trp = ps_sm.tile([C, C], BF16, tag="sm")
                nc.tensor.transpose(trp, KT[:, C * c:C * (c + 1)], s_ident)
                nc.scalar.activation(ktok[:, C * c:C * (c + 1)], trp, AF.Copy)

            # ---- phase E: state from first half (zeros on half-0 cores) ----
            S = ps_s.tile([D, DV + 1], F32)
            for c in range(NCH):
                nc.tensor.matmul(S, phi_t[:, C * c:C * (c + 1)], s_vp[:, c, :],
                                 start=(c == 0), stop=False)

            # ---- phase F: own chunks ----
            for c in range(NCH):
                snap = wrk.tile([D, DV + 1], BF16, tag="snap")
                nc.scalar.activation(snap, S, AF.Copy)

                A = ps_sm.tile([C, C], F32, tag="sm")
                nc.tensor.matmul(A, KT[:, C * c:C * (c + 1)],
                                 QT[:, C * c:C * (c + 1)], start=True, stop=True)
                import concourse.mybir as mybir_
                Am = wrk.tile([C, C], BF16, tag="Am")
                nc.vector.tensor_tensor(out=Am, in0=A, in1=s_mask,
                                        op=mybir_.AluOpType.mult)

                O = ps_sm.tile([C, DV + 1], F32, tag="sm")
                nc.tensor.matmul(O, Am, s_v[:, c, :], start=True, stop=False)
                nc.tensor.matmul(O, QT[:, C * c:C * (c + 1)], snap,
                                 start=False, stop=True)

                nc.tensor.matmul(S, ktok[:, C * c:C * (c + 1)], s_v[:, c, :],
                                 start=False, stop=(c == NCH - 1))

                rec = wrk.tile([C, 1], F32, tag="rec")
                nc.vector.reciprocal(rec, O[:, DV:DV + 1])
                outc = wrk.tile([C, DV], F32, tag="outc")
                nc.scalar.activation(outc, O[:, 0:DV], AF.Copy,
                                     bias=0.0, scale=rec)
                nc.sync.dma_start(out=out[C * c:C * (c + 1), :], in_=outc)

    nc.compile()
    return nc


def _get_nc():
    if "nc" not in _cache:
        _cache["nc"] = _build()
    return _cache["nc"]


def kernel(q, k, v, W_phi, b_phi):
    import ml_dtypes
    from concourse.bass_utils import run_bass_kernel_spmd

    bf16 = ml_dtypes.bfloat16
    q = np.asarray(q, np.float32)
    k = np.asarray(k, np.float32)
    v = np.asarray(v, np.float32)
    W_phi = np.asarray(W_phi, np.float32)
    b_phi = np.asarray(b_phi, np.float32)

    WT = np.ascontiguousarray(W_phi.T)                    # [d, e]
    bias = b_phi.reshape(D, 1)
    btile = np.tile(b_phi, NCH).reshape(1, H)             # chunk-major bias
    maskm = np.triu(np.ones((C, C), np.float32))          # keep tau <= t
    ident = np.eye(C, dtype=np.float32)

    def aug(vh):  # [H, DV] -> [NCH, C, DV+1] with ones column
        a = np.concatenate([vh, np.ones((H, 1), np.float32)], axis=1)
        return a.reshape(NCH, C, DV + 1).astype(bf16)

    zeros_kT = np.zeros((D, H), dtype=bf16)
    zeros_v = np.zeros((NCH, C, DV + 1), dtype=bf16)

    in_maps = []
    for core in range(NCORES):
        b_idx, half = divmod(core, 2)
        sl = slice(half * H, (half + 1) * H)
        m = {
            "qT": np.ascontiguousarray(q[b_idx, sl].T),
            "kT": np.ascontiguousarray(k[b_idx, sl].T),
            "WT": WT,
            "WTb": WT.astype(bf16),
            "bias": bias,
            "btile": btile.astype(bf16),
            "vaug": aug(v[b_idx, sl]),
            "mask": maskm.astype(bf16),
            "ident": ident.astype(bf16),
        }
        if half == 1:
            m["kTp"] = np.ascontiguousarray(k[b_idx, 0:H].T).astype(bf16)
            m["vaugp"] = aug(v[b_idx, 0:H])
        else:
            m["kTp"] = zeros_kT
            m["vaugp"] = zeros_v
        in_maps.append(m)

    nc = _get_nc()
    res = run_bass_kernel_spmd(nc, in_maps, list(range(NCORES)))

    out = np.empty((B, T, DV), np.float32)
    for core in range(NCORES):
        b_idx, half = divmod(core, 2)
        out[b_idx, half * H:(half + 1) * H] = res.results[core]["out"]
    return out


# revision 5
# speedup vs baseline: 1.1028x; 1.1028x over previous
"""Causal linear attention (ELU+1 feature map) on 8 trn2 NeuronCores.

Sharding: core i handles batch b=i//2, sequence half h=i%2 (T=2048 -> 1024
tokens/core).  Second-half cores recompute the first half's running state
S0 = sum_tau phi(k_tau) [v_tau, 1]  (128x129, col 128 = z) from k/v of the
first half; first-half cores get zeroed aux inputs so their S0 == 0.  This
avoids any cross-core communication.

Math per core (chunk C=128, 8 own chunks + 8 "pre" state-only chunks):
  phi(y) = min(exp(y), 1) + relu(y)            (== ELU(y)+1 exactly)
  pre_q/pre_k = W_phi @ q/k (feature-major, fp32r matmuls)
  pre_tok(kpre) = token-major via per-chunk stationary kT + rank-1 bias matmul
  A^T_c = K_c Q_c^T ; mask (tau<=t) fused into PSUM->SBUF bf16 copy
  O_c = Amask^T.T @ [V_c, 1] + Q_c @ S_snap    (den accumulates in col 128)
  S  += K_tok_c^T-style matmul with [V_c, 1]   (PSUM accumulation)
  out_c = O_c[:, :128] * (1 / O_c[:, 128])

Inputs are packed into two big DMAs (one f32, one bf16) issued via the idle
Pool/SWDGE path; output is a single staged DMA.  Host packs/unpacks.
"""

import numpy as np

B, T, D, DV = 4, 2048, 128, 128
H = T // 2          # tokens per core
C = 128             # chunk
NCH = H // C        # chunks per half
NCORES = 8

# f32 input pack columns: [qT | kT | WT | bias]
F32_COLS = H + H + D + 1
OFF_QT, OFF_KT, OFF_WT, OFF_BIAS = 0, H, 2 * H, 2 * H + D
# bf16 input pack columns: [kTp | v (NCH x 129) | vp | WTb | mask | ident]
VW = DV + 1
B16_COLS = H + NCH * VW + NCH * VW + D + C + C
OFF_KTP = 0
OFF_V = H
OFF_VP = H + NCH * VW
OFF_WTB = OFF_VP + NCH * VW
OFF_MASK = OFF_WTB + D
OFF_ID = OFF_MASK + C

_cache = {}


def _build():
    import concourse.bacc as bacc
    import concourse.tile as tile
    from concourse import mybir

    F32 = mybir.dt.float32
    F32R = mybir.dt.float32r
    BF16 = mybir.dt.bfloat16
    AF = mybir.ActivationFunctionType
    MUL = mybir.AluOpType.mult

    nc = bacc.Bacc(None, target_bir_lowering=False, debug=False,
                   num_devices=NCORES)

    fin = nc.declare_dram_parameter("fin", [D, F32_COLS], F32R, isOutput=False)
    bin_ = nc.declare_dram_parameter("bin", [D, B16_COLS], BF16, isOutput=False)
    btile = nc.declare_dram_parameter("btile", [1, H], BF16, isOutput=False)
    out = nc.declare_dram_parameter("out", [C, NCH * DV], F32, isOutput=True)

    with tile.TileContext(nc) as tc:
        with (
            tc.tile_pool(name="cst", bufs=1) as cst,
            tc.tile_pool(name="io", bufs=1) as io,
            tc.tile_pool(name="phi", bufs=1) as phip,
            tc.tile_pool(name="wrk", bufs=3) as wrk,
            tc.tile_pool(name="ps_pre", bufs=2, space="PSUM") as ps_pre,
            tc.tile_pool(name="ps_s", bufs=1, space="PSUM") as ps_s,
            tc.tile_pool(name="ps_sm", bufs=3, space="PSUM") as ps_sm,
        ):
            # ---- warm the ACT table while DMAs run ----
            s_warm = cst.tile([D, 1], F32)
            nc.vector.memset(s_warm, 0.0)
            s_warm2 = cst.tile([D, 1], BF16)
            nc.scalar.activation(s_warm2, s_warm, AF.Exp)

            # ---- loads: two big input DMAs on Pool/SWDGE, btile on sync ----
            s_fin = io.tile([D, F32_COLS], F32R)
            s_b16 = io.tile([D, B16_COLS], BF16)
            s_btile = cst.tile([1, H], BF16)
            s_ones = cst.tile([1, C], BF16)
            nc.gpsimd.dma_start(out=s_fin, in_=fin[:, :])
            nc.gpsimd.dma_start(out=s_b16, in_=bin_[:, :])
            nc.sync.dma_start(out=s_btile, in_=btile[:, :])
            nc.vector.memset(s_ones, 1.0)

            s_bias = cst.tile([D, 1], F32)
            nc.vector.tensor_copy(s_bias, s_fin[:, OFF_BIAS:OFF_BIAS + 1])

            sWT = s_fin[:, OFF_WT:OFF_WT + D]
            sWTb = s_b16[:, OFF_WTB:OFF_WTB + D]
            s_mask = s_b16[:, OFF_MASK:OFF_MASK + C]
            s_ident = s_b16[:, OFF_ID:OFF_ID + C]

            def vsl(c):
                return s_b16[:, OFF_V + VW * c:OFF_V + VW * (c + 1)]

            def vpsl(c):
                return s_b16[:, OFF_VP + VW * c:OFF_VP + VW * (c + 1)]

            # ---- token-major pre for K_pre (state recompute path first) ----
            pst = ps_pre.tile([C, H], F32, tag="pre")
            for j in range(H // 512):
                nc.tensor.matmul(pst[:, 512 * j:512 * (j + 1)], s_ones,
                                 s_btile[:, 512 * j:512 * (j + 1)],
                                 start=True, stop=True)
            for c in range(NCH):
                nc.tensor.matmul(pst[:, C * c:C * (c + 1)],
                                 s_b16[:, OFF_KTP + C * c:OFF_KTP + C * (c + 1)],
                                 sWTb, start=False, stop=True)
            e_t = phip.tile([C, H], BF16)
            r_t = phip.tile([C, H], BF16)
            phi_t = phip.tile([C, H], BF16)       # K_tok_pre chunks
            nc.scalar.activation(e_t, pst, AF.Exp)
            nc.scalar.activation(r_t, pst, AF.Relu)
            nc.vector.tensor_scalar_min(e_t, e_t, 1.0)
            nc.vector.tensor_add(phi_t, e_t, r_t)

            # state accumulator; pre-half contributions (zeros on half-0)
            S = ps_s.tile([D, DV + 1], F32)
            for c in range(NCH):
                nc.tensor.matmul(S, phi_t[:, C * c:C * (c + 1)], vpsl(c),
                                 start=(c == 0), stop=False)

            # ---- feature-major phi for own q, k ----
            e_f = phip.tile([D, 2 * H], BF16)
            r_f = phip.tile([D, 2 * H], BF16)
            phi_f = phip.tile([D, 2 * H], BF16)   # [Q^T | K^T]
            for i, off in enumerate((OFF_QT, OFF_KT)):
                pre = ps_pre.tile([D, H], F32, tag="pre")
                for j in range(H // 512):
                    nc.tensor.matmul(pre[:, 512 * j:512 * (j + 1)], sWT,
                                     s_fin[:, off + 512 * j:off + 512 * (j + 1)],
                                     start=True, stop=True)
                sl = slice(H * i, H * (i + 1))
                nc.scalar.activation(e_f[:, sl], pre, AF.Exp,
                                     bias=s_bias, scale=1.0)
                nc.scalar.activation(r_f[:, sl], pre, AF.Relu,
                                     bias=s_bias, scale=1.0)
                nc.vector.tensor_scalar_min(e_f[:, sl], e_f[:, sl], 1.0)
                nc.vector.tensor_add(phi_f[:, sl], e_f[:, sl], r_f[:, sl])

            QT = phi_f[:, 0:H]
            KT = phi_f[:, H:2 * H]

            # ---- transpose own K chunks (feature->token major) ----
            ktok = phip.tile([C, H], BF16)
            for c in range(NCH):
                trp = ps_sm.tile([C, C], BF16, tag="sm")
                nc.tensor.transpose(trp, KT[:, C * c:C * (c + 1)], s_ident)
                nc.vector.tensor_copy(ktok[:, C * c:C * (c + 1)], trp)

            # ---- own chunks ----
            outstage = phip.tile([C, NCH * DV], F32)
            for c in range(NCH):
                snap = wrk.tile([D, DV + 1], BF16, tag="snap")
                nc.scalar.activation(snap, S, AF.Copy)

                A = ps_sm.tile([C, C], F32, tag="sm")
                nc.tensor.matmul(A, KT[:, C * c:C * (c + 1)],
                                 QT[:, C * c:C * (c + 1)], start=True, stop=True)
                Am = wrk.tile([C, C], BF16, tag="Am")
                nc.vector.tensor_tensor(out=Am, in0=A, in1=s_mask, op=MUL)

                O = ps_sm.tile([C, DV + 1], F32, tag="sm")
                nc.tensor.matmul(O, Am, vsl(c), start=True, stop=False)
                nc.tensor.matmul(O, QT[:, C * c:C * (c + 1)], snap,
                                 start=False, stop=True)

                nc.tensor.matmul(S, ktok[:, C * c:C * (c + 1)], vsl(c),
                                 start=False, stop=(c == NCH - 1))

                rec = wrk.tile([C, 1], F32, tag="rec")
                nc.vector.reciprocal(rec, O[:, DV:DV + 1])
                nc.scalar.activation(outstage[:, DV * c:DV * (c + 1)],
                                     O[:, 0:DV], AF.Copy, bias=0.0, scale=rec)

            nc.sync.dma_start(out=out[:, :], in_=outstage)

    nc.compile()
    return nc


def _get_nc():
    if "nc" not in _cache:
        _cache["nc"] = _build()
    return _cache["nc"]


def _pack_inputs(q, k, v, W_phi, b_phi):
    import ml_dtypes
    bf16 = ml_dtypes.bfloat16

    WT = np.ascontiguousarray(W_phi.T)                    # [d, e]
    maskm = np.triu(np.ones((C, C), np.float32))          # keep tau <= t
    ident = np.eye(C, dtype=np.float32)
    btile = np.tile(b_phi, NCH).reshape(1, H).astype(bf16)

    def aug(vh):  # [H, DV] -> [C, NCH*(DV+1)] partition-major with ones col
        a = np.concatenate([vh, np.ones((H, 1), np.float32)], axis=1)
        return a.reshape(NCH, C, VW).transpose(1, 0, 2).reshape(C, NCH * VW)

    zeros_vp = np.zeros((C, NCH * VW), np.float32)
    zeros_ktp = np.zeros((D, H), np.float32)

    in_maps = []
    for core in range(NCORES):
        b_idx, half = divmod(core, 2)
        sl = slice(half * H, (half + 1) * H)
        fin = np.empty((D, F32_COLS), np.float32)
        fin[:, OFF_QT:OFF_QT + H] = q[b_idx, sl].T
        fin[:, OFF_KT:OFF_KT + H] = k[b_idx, sl].T
        fin[:, OFF_WT:OFF_WT + D] = WT
        fin[:, OFF_BIAS] = b_phi
        b16 = np.empty((D, B16_COLS), np.float32)
        if half == 1:
            b16[:, OFF_KTP:OFF_KTP + H] = k[b_idx, 0:H].T
            b16[:, OFF_VP:OFF_VP + NCH * VW] = aug(v[b_idx, 0:H])
        else:
            b16[:, OFF_KTP:OFF_KTP + H] = zeros_ktp
            b16[:, OFF_VP:OFF_VP + NCH * VW] = zeros_vp
        b16[:, OFF_V:OFF_V + NCH * VW] = aug(v[b_idx, sl])
        b16[:, OFF_WTB:OFF_WTB + D] = WT
        b16[:, OFF_MASK:OFF_MASK + C] = maskm
        b16[:, OFF_ID:OFF_ID + C] = ident
        in_maps.append({"fin": fin, "bin": b16.astype(bf16), "btile": btile})
    return in_maps


def kernel(q, k, v, W_phi, b_phi):
    from concourse.bass_utils import run_bass_kernel_spmd

    q = np.asarray(q, np.float32)
    k = np.asarray(k, np.float32)
    v = np.asarray(v, np.float32)
    W_phi = np.asarray(W_phi, np.float32)
    b_phi = np.asarray(b_phi, np.float32)

    in_maps = _pack_inputs(q, k, v, W_phi, b_phi)
    nc = _get_nc()
    res = run_bass_kernel_spmd(nc, in_maps, list(range(NCORES)))

    out = np.empty((B, T, DV), np.float32)
    for core in range(NCORES):
        b_idx, half = divmod(core, 2)
        o = res.results[core]["out"]                      # [C, NCH*DV]
        o = o.reshape(C, NCH, DV).transpose(1, 0, 2).reshape(H, DV)
        out[b_idx, half * H:(half + 1) * H] = o
    return out
